# revision 1
# baseline (speedup 1.0000x reference)
"""DyGNN streaming-interaction kernel for Trainium2 (8 NeuronCores, Bass/Tile).

Strategy
--------
The reference is a sequential scan over S=2048 events touching rows of five
[N=100000, 128] node-state tables.  The output is only the PRE-update node
representation gathered at each event, so an event's update math matters only
if a LATER event reads one of its two nodes.  With random indices that is a
small set ("relevant" events, ~82 for the expected data) with a very shallow
dependency depth (~2 levels).

Host side (index math only): find relevant events, batch them into dependency
levels, compute operand provenance, and route the 2*S output-row gathers to
the core owning each node (node_rep is sharded row-wise across the 8 cores).

Device side (single SPMD program, per-core data):
  * each core gathers its share of output rows from its node_rep shard with
    one multi-row indirect DMA and writes them out contiguously;
  * the relevant-event recurrence (edge updaters + time-decayed LSTMs +
    combiner) runs as batched fp32 matmuls in a transposed
    [feature, head-events | tail-events] layout, one batch per dependency
    level (replicated on all cores - it is tiny - core 0's result is used).
    Sigmoid is computed as 0.5 + 0.5*tanh(x/2) so the whole kernel uses a
    single ACT table set (exp+tanh) - no table-switch stalls.

Host side assembles the [2, S, D] output from the per-core gather buffers
plus the computed representations for the few "patched" slots.
"""

import numpy as np

_NCORES = 8
_D = 128          # embedding dim == partition count
_MAXB = 256       # max events per device batch ([128, 2B] fits one PSUM bank)
_MAX_LEVELS = 64  # beyond this (adversarial chains) use the host fallback
_W_DECAY = 1.0

# operand order inside the packed per-level seed tile [128, 10*B]:
# RH RT DTH DTT | CH CT | HH HT | XHT XHH   (each block is B columns; the
# first four blocks feed the chain head - edges and decay - and ship first)
_OP_RH, _OP_RT, _OP_DTH, _OP_DTT, _OP_CH, _OP_CT, _OP_HH, _OP_HT, _OP_XHT, _OP_XHH = range(10)
_N_OPS = 10

_cache = {}
last_result = None  # BassKernelResults of the most recent device run


def _preprocess(heads, tails, times):
    """Pure index/time analysis.  Returns None if the dependency structure is
    too deep for the compiled-levels approach (host fallback handles it).

    Level-1 events read only the initial tables and have no intra-level
    dependencies, so they are SPLIT across the 8 cores.  Level-1 events whose
    results feed later levels ("feeders"), and all events of level >= 2, are
    pinned to core 0 so every result-to-operand copy stays core-local (the
    SPMD program is identical on every core; the other cores run the deeper
    levels on zero padding and their results are ignored).
    """
    S = heads.shape[0]

    # -- pass 1 (backward): does any later event touch this event's nodes? --
    touched_later = np.zeros(S, dtype=bool)
    seen = {}
    for i in range(S - 1, -1, -1):
        h = int(heads[i]); t = int(tails[i])
        touched_later[i] = (h in seen) or (t in seen)
        seen[h] = True; seen[t] = True
    rel = [i for i in range(S) if touched_later[i]]

    # -- pass 2: dependency levels (global width cap keeps compile sane) --
    level_of = {}
    level_events = []
    last_level = {}
    for i in rel:
        h = int(heads[i]); t = int(tails[i])
        lv = max(last_level.get(h, 0), last_level.get(t, 0)) + 1
        while lv - 1 < len(level_events) and len(level_events[lv - 1]) >= _MAXB:
            lv += 1
        if lv > _MAX_LEVELS:
            return None
        while len(level_events) < lv:
            level_events.append([])
        level_of[i] = lv - 1
        level_events[lv - 1].append(i)
        last_level[h] = lv; last_level[t] = lv
    L = len(level_events)

    # -- pass 3 (forward): per-event operand sources (event-id based) --
    sources = {}   # i -> list of (op_idx, src_event|None, src_kind|table, node)
    dts = {}       # i -> (dt_h, dt_t)
    lastw = {"rep": {}, "ch": {}, "hh": {}, "ct": {}, "ht": {}}
    last_time = {}
    feeders = set()
    for i in rel:
        h = int(heads[i]); t = int(tails[i]); tm = np.float32(times[i])
        srcs = []
        for op_idx, key, table, node in (
            (_OP_RH, "rep", "node_rep", h),
            (_OP_RT, "rep", "node_rep", t),
            (_OP_CH, "ch", "cell_head", h),
            (_OP_CT, "ct", "cell_tail", t),
            (_OP_HH, "hh", "hidden_head", h),
            (_OP_HT, "ht", "hidden_tail", t),
            (_OP_XHT, "ht", "hidden_tail", h),
            (_OP_XHH, "hh", "hidden_head", t),
        ):
            src = lastw[key].get(node)
            if src is not None:
                feeders.add(src[0])
            srcs.append((op_idx, src, table, node))
        sources[i] = srcs
        dts[i] = (np.float32(tm - np.float32(last_time.get(h, 0.0))),
                  np.float32(tm - np.float32(last_time.get(t, 0.0))))
        lastw["rep"][h] = (i, "NRH")
        lastw["rep"][t] = (i, "NRT")
        lastw["ch"][h] = (i, "CHN")
        lastw["hh"][h] = (i, "HHN")
        lastw["ct"][t] = (i, "CTN")
        lastw["ht"][t] = (i, "HTN")
        last_time[h] = tm; last_time[t] = tm

    # -- pass 4: core assignment.  Feeders + all level>=2 events -> core 0;
    # remaining level-1 events spread greedily across all cores. --
    chunks = [[[] for _ in range(L)] for _ in range(_NCORES)]  # [core][lv]
    assign = {}
    free_l1 = []
    for i in rel:
        lv = level_of[i]
        if lv > 0 or i in feeders:
            assign[i] = (0, lv, len(chunks[0][lv]))
            chunks[0][lv].append(i)
        else:
            free_l1.append(i)
    if L:
        loads = [len(chunks[k][0]) for k in range(_NCORES)]
        for i in free_l1:
            k = int(np.argmin(loads))
            assign[i] = (k, 0, len(chunks[k][0]))
            chunks[k][0].append(i)
            loads[k] += 1

    Bs = [max(len(chunks[k][l]) for k in range(_NCORES)) for l in range(L)]
    off = [0]
    for b in Bs:
        off.append(off[-1] + 2 * b)
    Ctot = off[-1]

    # -- pass 5: program copies (core-0 positions; identical on all cores)
    # and per-core seed fills --
    copies = [[] for _ in range(L)]
    seed_fill = [[[] for _ in range(L)] for _ in range(_NCORES)]
    dt_fill = [[[] for _ in range(L)] for _ in range(_NCORES)]
    for i in rel:
        k, lv, p = assign[i]
        B = Bs[lv]
        for (op_idx, src, table, node) in sources[i]:
            dst_col = op_idx * B + p
            if src is not None:
                j, skind = src
                sk, slv, sp = assign[j]
                # feeders and deep levels are all on core 0, as is event i
                assert sk == 0 and k == 0
                stile = {"CHN": "C", "CTN": "C", "HHN": "H", "HTN": "H",
                         "NRH": "R", "NRT": "R"}[skind]
                s_col = sp if skind in ("CHN", "HHN", "NRH") else Bs[slv] + sp
                copies[lv].append((dst_col, slv, stile, s_col))
            else:
                seed_fill[k][lv].append((dst_col, table, node))
        dt_fill[k][lv].append((p, dts[i][0], dts[i][1]))

    return {
        "touched_later": touched_later,
        "assign": assign,
        "Bs": Bs,
        "off": off,
        "Ctot": Ctot,
        "copies": copies,
        "seed_fill": seed_fill,
        "dt_fill": dt_fill,
    }


def _route_outputs(heads, tails, plan, N):
    """Route each of the 2*S output slots to either a per-core gather or a
    computed-rep column."""
    S = heads.shape[0]
    Bs, off, assign = plan["Bs"], plan["off"], plan["assign"]
    touched_later = plan["touched_later"]
    shard = -(-N // _NCORES)  # ceil

    gl_idx = [[] for _ in range(_NCORES)]
    gl_slot = [[] for _ in range(_NCORES)]
    comp_slots = [[] for _ in range(_NCORES)]  # per compute-owner core
    comp_cols = [[] for _ in range(_NCORES)]
    lastw_rep_col = {}
    for i in range(S):
        h = int(heads[i]); t = int(tails[i])
        for role, n in ((0, h), (1, t)):
            slot = role * S + i
            cc = lastw_rep_col.get(n)
            if cc is not None:
                comp_slots[cc[0]].append(slot); comp_cols[cc[0]].append(cc[1])
            else:
                k = n // shard
                gl_idx[k].append(n - k * shard)
                gl_slot[k].append(slot)
        if touched_later[i]:
            k, lv, p = assign[i]
            lastw_rep_col[h] = (k, off[lv] + p)           # NRH column
            lastw_rep_col[t] = (k, off[lv] + Bs[lv] + p)  # NRT column

    max_load = max(len(x) for x in gl_idx)
    G = max(1, -(-max_load // 128))
    n_pad = G * 128
    oidx = []
    for k in range(_NCORES):
        a = np.zeros(n_pad, dtype=np.int32)
        a[: len(gl_idx[k])] = gl_idx[k]
        # gathered row g*128+p comes from SBUF [p, g*128:(g+1)*128]
        oidx.append(np.ascontiguousarray(a.reshape(G, 128).T))
    return {
        "shard": shard,
        "G": G,
        "n_pad": n_pad,
        "oidx": oidx,
        "gl_slot": gl_slot,
        "comp_slots": [np.array(x, dtype=np.int64) for x in comp_slots],
        "comp_cols": [np.array(x, dtype=np.int64) for x in comp_cols],
    }


# packed weight layout (name -> column width), ordered by first use; the
# device stages each weight with its own slice DMA so early matmuls do not
# wait for the whole pack
_WPACK = (("Weh1", 128), ("Weh2", 128), ("Wet1", 128), ("Wet2", 128),
          ("Wdh", 128), ("Wdt", 128)) + tuple(
    (f"{m}{q}", 128) for q in range(4) for m in ("Wxh", "Whh", "Wxt", "Wht")
) + (("Wc1", 128), ("Wc2", 128))
_BPACK = (("beh", 1), ("bet", 1), ("bdh", 1), ("bdt", 1),
          ("bh4h", 4), ("bt4h", 4), ("bh4", 4), ("bt4", 4))


def _wcols(pack):
    offs, o = {}, 0
    for name, w in pack:
        offs[name] = (o, w)
        o += w
    return offs, o


def _build_program(shard, G, Bs, copies, Ctot, has_bias):
    from contextlib import ExitStack

    import concourse.bacc as bacc
    import concourse.bass as bass
    import concourse.tile as tile
    from concourse import mybir

    f32 = mybir.dt.float32
    i32 = mybir.dt.int32
    AFT = mybir.ActivationFunctionType

    nc = bacc.Bacc(
        "TRN2",
        debug=False,
        enable_asserts=False,
        target_bir_lowering=False,
        num_devices=_NCORES,
        enable_partition_id=False,
    )

    rep = nc.dram_tensor("rep", [shard, _D], f32, kind="ExternalInput").ap()
    oidx = nc.dram_tensor("oidx", [128, G], i32, kind="ExternalInput").ap()
    out_gath = nc.dram_tensor("out_gath", [128, G * _D], f32, kind="ExternalOutput").ap()

    L = len(Bs)
    offsW, WW = _wcols(_WPACK)
    offsBias, WBias = _wcols(_BPACK)
    seeds_dram = []
    comp = None
    if L:
        wpack = nc.dram_tensor("wpack", [128, WW], f32, kind="ExternalInput").ap()
        if has_bias:
            bpack = nc.dram_tensor("bpack", [128, WBias], f32, kind="ExternalInput").ap()
        for l, B in enumerate(Bs):
            seeds_dram.append(
                nc.dram_tensor(f"seeds{l}", [128, _N_OPS * B], f32, kind="ExternalInput").ap()
            )
        comp = nc.dram_tensor("comp", [128, Ctot], f32, kind="ExternalOutput").ap()

    with tile.TileContext(nc) as tc, ExitStack() as ctx:
        wp = ctx.enter_context(tc.tile_pool(name="w", bufs=1))
        lp = ctx.enter_context(tc.tile_pool(name="lv", bufs=1))
        tp = ctx.enter_context(tc.tile_pool(name="tmp", bufs=3))
        pp = ctx.enter_context(tc.tile_pool(name="ps", bufs=8, space="PSUM"))
        gp = ctx.enter_context(tc.tile_pool(name="g", bufs=1))

        # ---- staging DMAs ----
        if L:
            sd_tiles = []
            for l, B in enumerate(Bs):
                t = lp.tile([128, _N_OPS * B], f32, tag=f"sd{l}", name=f"sd{l}")
                sd_tiles.append(t)
            wt = wp.tile([128, WW], f32, tag="wt", name="wt")

            def w(name):
                o, wd = offsW[name]
                return wt[:, o : o + wd]

            # Few, coarse staging DMAs in first-use order, split across the
            # two HWDGE rings so no completion-semaphore lane entangles an
            # early consumer with a big later transfer.
            wsplit = offsW["Wxh0"][0]  # edge/cs weights end here
            wcomb = offsW["Wc1"][0]    # gate weights end here
            nc.sync.dma_start(sd_tiles[0][:], seeds_dram[0][:])
            nc.sync.dma_start(wt[:, :wsplit], wpack[:, :wsplit])
            nc.sync.dma_start(wt[:, wsplit:wcomb], wpack[:, wsplit:wcomb])
            nc.sync.dma_start(wt[:, wcomb:], wpack[:, wcomb:])
            if has_bias:
                bt_ = wp.tile([128, WBias], f32, tag="bias", name="biasT")
                nc.sync.dma_start(bt_[:], bpack[:])
            for l in range(1, L):
                nc.sync.dma_start(sd_tiles[l][:], seeds_dram[l][:])

            def bias_ap(name, col=0):
                o, _ = offsBias[name]
                return bt_[:, o + col : o + col + 1]


        # ---- relevant-event recurrence ----
        results = []
        for l, B in enumerate(Bs) if L else []:
            B2 = 2 * B
            SD = sd_tiles[l]
            lvl_copies = sorted(copies[l], key=lambda c: c[2] == "R")
            for (dst_col, slv, stile, s_col) in lvl_copies:
                nc.vector.tensor_copy(
                    SD[:, dst_col : dst_col + 1],
                    results[slv][stile][:, s_col : s_col + 1],
                )
            blk = lambda op: SD[:, op * B : (op + 1) * B]
            pair = lambda op: SD[:, op * B : (op + 2) * B]
            RH, RT = blk(_OP_RH), blk(_OP_RT)
            C2 = pair(_OP_CH)
            HH, HT = blk(_OP_HH), blk(_OP_HT)
            XHT, XHH = blk(_OP_XHT), blk(_OP_XHH)
            DT2 = pair(_OP_DTH)

            def mm4(ps, wl1, rl1, wl2, rl2, wr1, rr1, wr2, rr2):
                # left block cols [0:B], right block cols [B:2B]; each block
                # accumulates two matmuls in PSUM
                nc.tensor.matmul(ps[:, 0:B], w(wl1), rl1, start=True, stop=wl2 is None)
                if wl2 is not None:
                    nc.tensor.matmul(ps[:, 0:B], w(wl2), rl2, start=False, stop=True)
                nc.tensor.matmul(ps[:, B:B2], w(wr1), rr1, start=True, stop=wr2 is None)
                if wr2 is not None:
                    nc.tensor.matmul(ps[:, B:B2], w(wr2), rr2, start=False, stop=True)
                return ps

            def act_pair(dst, src, func, scale=1.0, bias_l=None, bias_r=None):
                # one ACT across both blocks in the zero-bias fast path,
                # else one per block with its per-partition bias
                if not has_bias or (bias_l is None and bias_r is None):
                    nc.scalar.activation(dst[:], src[:], func, scale=scale)
                else:
                    nc.scalar.activation(dst[:, 0:B], src[:, 0:B], func,
                                         bias=bias_l, scale=scale)
                    nc.scalar.activation(dst[:, B:B2], src[:, B:B2], func,
                                         bias=bias_r, scale=scale)
                return dst

            def tmp(tag):
                return tp.tile([128, B2], f32, tag=tag, name=f"t{l}_{tag}")

            # decay = exp(-w * dt)
            DEC = tmp("dec")
            nc.scalar.activation(DEC[:], DT2[:], AFT.Exp, scale=-_W_DECAY)

            # edges = tanh(rep_h @ We1 + rep_t @ We2 + be)
            EG = tmp("eg")
            ps_e = pp.tile([128, B2], f32, tag="ps", name=f"ps_e{l}")
            mm4(ps_e, "Weh1", RH, "Weh2", RT, "Wet1", RH, "Wet2", RT)
            act_pair(EG, ps_e, AFT.Tanh,
                     bias_l=bias_ap("beh") if has_bias else None,
                     bias_r=bias_ap("bet") if has_bias else None)

            # short-term memory cs = tanh(c @ Wd + bd); c_adj = c - cs + cs*dec
            CS = tmp("cs")
            ps_c = pp.tile([128, B2], f32, tag="ps", name=f"ps_c{l}")
            mm4(ps_c, "Wdh", C2[:, 0:B], None, None, "Wdt", C2[:, B:B2], None, None)
            act_pair(CS, ps_c, AFT.Tanh,
                     bias_l=bias_ap("bdh") if has_bias else None,
                     bias_r=bias_ap("bdt") if has_bias else None)
            CSD = tmp("csd")
            nc.vector.tensor_mul(CSD[:], CS[:], DEC[:])
            CMC = tmp("cmc")
            nc.vector.tensor_sub(CMC[:], C2[:], CS[:])
            CADJ = tmp("cadj")
            nc.vector.tensor_add(CADJ[:], CMC[:], CSD[:])

            # prefill combiner operand pairs with their seed-sourced halves
            U1 = tmp("u1")
            nc.vector.tensor_copy(U1[:, B:B2], XHH)
            U2 = tmp("u2")
            nc.vector.tensor_copy(U2[:, 0:B], XHT)

            # gates: z_q = edge @ Wx[:,q] + h @ Wh[:,q] + b[q]
            # sigmoid(z) computed as 0.5 + 0.5*tanh(z/2) (same ACT table set)
            gates = []
            for q in range(4):
                ps_q = pp.tile([128, B2], f32, tag="ps", name=f"ps_q{l}_{q}")
                # hidden-state terms first: they depend only on seeds (and
                # early copies), so the scheduler can hoist them into PE idle
                # bubbles before the edge activations are ready
                nc.tensor.matmul(ps_q[:, 0:B], w(f"Whh{q}"), HH, start=True, stop=False)
                nc.tensor.matmul(ps_q[:, 0:B], w(f"Wxh{q}"), EG[:, 0:B], start=False, stop=True)
                nc.tensor.matmul(ps_q[:, B:B2], w(f"Wht{q}"), HT, start=True, stop=False)
                nc.tensor.matmul(ps_q[:, B:B2], w(f"Wxt{q}"), EG[:, B:B2], start=False, stop=True)
                gq = tmp(f"g{q}")
                if q < 3:
                    act_pair(gq, ps_q, AFT.Tanh, scale=0.5,
                             bias_l=bias_ap("bh4h", q) if has_bias else None,
                             bias_r=bias_ap("bt4h", q) if has_bias else None)
                    import concourse.mybir as _mb
                    nc.vector.tensor_scalar(gq[:], gq[:], 0.5, 0.5,
                                            _mb.AluOpType.mult, _mb.AluOpType.add)
                else:
                    act_pair(gq, ps_q, AFT.Tanh,
                             bias_l=bias_ap("bh4", q) if has_bias else None,
                             bias_r=bias_ap("bt4", q) if has_bias else None)
                gates.append(gq)
            gi, gf, go, gg = gates

            # c_new = f*c_adj + i*g ; h_new = o*tanh(c_new)
            FC = tmp("fc")
            nc.vector.tensor_mul(FC[:], gf[:], CADJ[:])
            IG = tmp("ig")
            nc.vector.tensor_mul(IG[:], gi[:], gg[:])
            C_new = lp.tile([128, B2], f32, tag=f"res{l}_C", name=f"res{l}_C")
            nc.vector.tensor_add(C_new[:], FC[:], IG[:])
            TC = tmp("tc")
            nc.scalar.activation(TC[:], C_new[:], AFT.Tanh)
            last_level = l == len(Bs) - 1
            if last_level:
                # h_new feeds only the combiner here: write its halves
                # straight into the assembled operand pairs
                H_new = None
                nc.vector.tensor_mul(U1[:, 0:B], go[:, 0:B], TC[:, 0:B])
                nc.vector.tensor_mul(U2[:, B:B2], go[:, B:B2], TC[:, B:B2])
            else:
                H_new = lp.tile([128, B2], f32, tag=f"res{l}_H", name=f"res{l}_H")
                nc.vector.tensor_mul(H_new[:], go[:], TC[:])
                # combiner: finish assembling [h_h | hh[t]] and [ht[h] | h_t]
                # (seed halves were prefilled right after the seeds landed)
                nc.vector.tensor_copy(U1[:, 0:B], H_new[:, 0:B])
                nc.vector.tensor_copy(U2[:, B:B2], H_new[:, B:B2])
            ps_r = pp.tile([128, B2], f32, tag="ps", name=f"ps_r{l}")
            nc.tensor.matmul(ps_r[:], w("Wc1"), U1[:], start=True, stop=False)
            nc.tensor.matmul(ps_r[:], w("Wc2"), U2[:], start=False, stop=True)
            R_new = lp.tile([128, B2], f32, tag=f"res{l}_R", name=f"res{l}_R")
            nc.scalar.activation(R_new[:], ps_r[:], AFT.Tanh)
            results.append({"C": C_new, "H": H_new, "R": R_new})

            o0 = sum(2 * b for b in Bs[:l])
            nc.scalar.dma_start(comp[:, o0 : o0 + B2], R_new[:])


        # ---- output-row gather (not on the critical path: emitted last so
        # ---- its DMA traffic stays clear of the startup staging window)
        idx_sb = gp.tile([128, G], i32, tag="idx", name="idx_sb")
        nc.sync.dma_start(idx_sb[:], oidx[:])
        gt = gp.tile([128, G * _D], f32, tag="gath", name="gt")
        for g in range(G):
            sl = slice(g * _D, (g + 1) * _D)
            nc.gpsimd.indirect_dma_start(
                out=gt[:, sl],
                out_offset=None,
                in_=rep[:],
                in_offset=bass.IndirectOffsetOnAxis(ap=idx_sb[:, g : g + 1], axis=0),
            )
            nc.sync.dma_start(out_gath[:, sl], gt[:, sl])

    nc.compile()
    return nc


def _pack_weight_arrays(inputs, has_bias):
    f32 = np.float32

    def pack(names_widths, arrs):
        cols = sum(w for _, w in names_widths)
        out = np.empty((128, cols), f32)
        o = 0
        for name, wd in names_widths:
            out[:, o : o + wd] = arrs[name]
            o += wd
        return out

    arrs = {}
    for n, _ in _WPACK:
        if n[-1] in "0123" and n[:-1] in ("Wxh", "Whh", "Wxt", "Wht"):
            q = int(n[-1])
            arrs[n] = np.asarray(inputs[n[:-1]], f32)[:, q * 128 : (q + 1) * 128]
        else:
            arrs[n] = np.asarray(inputs[n], f32)
    res = {"wpack": pack(_WPACK, arrs)}
    if has_bias:
        bh4 = np.asarray(inputs["bh"], f32).reshape(4, 128).T
        bt4 = np.asarray(inputs["bt"], f32).reshape(4, 128).T
        arrs = {
            "beh": np.asarray(inputs["beh"], f32).reshape(128, 1),
            "bet": np.asarray(inputs["bet"], f32).reshape(128, 1),
            "bdh": np.asarray(inputs["bdh"], f32).reshape(128, 1),
            "bdt": np.asarray(inputs["bdt"], f32).reshape(128, 1),
            "bh4h": 0.5 * bh4, "bt4h": 0.5 * bt4, "bh4": bh4, "bt4": bt4,
        }
        res["bpack"] = pack(_BPACK, arrs)
    return res


def _numpy_fallback(heads, tails, times, node_rep, cell_head, hidden_head,
                    cell_tail, hidden_tail, Weh1, Weh2, beh, Wet1, Wet2, bet,
                    Wxh, Whh, bh, Wdh, bdh, Wxt, Wht, bt, Wdt, bdt, Wc1, Wc2):
    """Exact float32 reference semantics; safety net for pathological inputs."""
    f32 = np.float32
    S = heads.shape[0]
    D = node_rep.shape[1]
    rep = np.array(node_rep, f32); ch = np.array(cell_head, f32)
    hh = np.array(hidden_head, f32); ct = np.array(cell_tail, f32)
    ht = np.array(hidden_tail, f32)
    rt = np.zeros(node_rep.shape[0], f32)
    out = np.zeros((2, S, D), f32)

    def sig(x):
        return f32(1.0) / (f32(1.0) + np.exp(-x, dtype=f32))

    def tlstm(x, c, h, dec, Wx, Wh, b, Wd, bd):
        cs = np.tanh(c @ Wd + bd, dtype=f32)
        c_adj = c - cs + cs * dec
        z = x @ Wx + h @ Wh + b
        i, f, o, g = np.split(z, 4)
        i = sig(i); f = sig(f); o = sig(o); g = np.tanh(g, dtype=f32)
        c_new = f * c_adj + i * g
        return c_new, o * np.tanh(c_new, dtype=f32)

    for j in range(S):
        h_i = int(heads[j]); t_i = int(tails[j]); tm = f32(times[j])
        rep_h = rep[h_i].copy(); rep_t = rep[t_i].copy()
        out[0, j] = rep_h; out[1, j] = rep_t
        dec_h = np.exp(f32(-_W_DECAY) * (tm - rt[h_i]), dtype=f32)
        dec_t = np.exp(f32(-_W_DECAY) * (tm - rt[t_i]), dtype=f32)
        edge_h = np.tanh(rep_h @ Weh1 + rep_t @ Weh2 + beh, dtype=f32)
        edge_t = np.tanh(rep_h @ Wet1 + rep_t @ Wet2 + bet, dtype=f32)
        c_h, h_h = tlstm(edge_h, ch[h_i], hh[h_i], dec_h, Wxh, Whh, bh, Wdh, bdh)
        c_t, h_t = tlstm(edge_t, ct[t_i], ht[t_i], dec_t, Wxt, Wht, bt, Wdt, bdt)
        new_rep_h = np.tanh(h_h @ Wc1 + ht[h_i] @ Wc2, dtype=f32)
        new_rep_t = np.tanh(hh[t_i] @ Wc1 + h_t @ Wc2, dtype=f32)
        rep[h_i] = new_rep_h; rep[t_i] = new_rep_t
        ch[h_i] = c_h; hh[h_i] = h_h; ct[t_i] = c_t; ht[t_i] = h_t
        rt[h_i] = tm; rt[t_i] = tm
    return out


def kernel(**inputs):
    global last_result
    heads = np.asarray(inputs["heads"]).astype(np.int64)
    tails = np.asarray(inputs["tails"]).astype(np.int64)
    times = np.asarray(inputs["times"], dtype=np.float32)
    node_rep = np.asarray(inputs["node_rep"], dtype=np.float32)
    N = node_rep.shape[0]
    S = heads.shape[0]

    plan = _preprocess(heads, tails, times)
    if plan is None:
        return _numpy_fallback(
            heads, tails, times,
            *[np.asarray(inputs[k], np.float32) for k in (
                "node_rep", "cell_head", "hidden_head", "cell_tail", "hidden_tail",
                "Weh1", "Weh2", "beh", "Wet1", "Wet2", "bet",
                "Wxh", "Whh", "bh", "Wdh", "bdh", "Wxt", "Wht", "bt", "Wdt", "bdt",
                "Wc1", "Wc2")],
        )

    routing = _route_outputs(heads, tails, plan, N)
    shard, G, n_pad = routing["shard"], routing["G"], routing["n_pad"]
    Bs, Ctot = plan["Bs"], plan["Ctot"]
    L = len(Bs)

    has_bias = bool(L) and any(
        np.any(np.asarray(inputs[k], np.float32))
        for k in ("beh", "bet", "bdh", "bdt", "bh", "bt"))

    sig = (shard, G, tuple(Bs),
           tuple(tuple(c) for lc in plan["copies"] for c in lc), Ctot, has_bias)
    nc = _cache.get(sig)
    if nc is None:
        nc = _build_program(shard, G, Bs, plan["copies"], Ctot, has_bias)
        _cache[sig] = nc

    # per-core, per-level packed seed tensors [128, 10*B] from the tables
    tables = {k: np.asarray(inputs[k], np.float32) for k in (
        "node_rep", "cell_head", "hidden_head", "cell_tail", "hidden_tail")}
    seeds = [[np.zeros((128, _N_OPS * B), np.float32) for B in Bs]
             for _ in range(_NCORES)]
    for k in range(_NCORES):
        for l, B in enumerate(Bs):
            for (col, tab, node) in plan["seed_fill"][k][l]:
                seeds[k][l][:, col] = tables[tab][node]
            for (p, dt_h, dt_t) in plan["dt_fill"][k][l]:
                seeds[k][l][:, _OP_DTH * B + p] = dt_h
                seeds[k][l][:, _OP_DTT * B + p] = dt_t

    w_in = _pack_weight_arrays(inputs, has_bias) if L else {}

    pad_rows = shard * _NCORES - N
    rep_padded = node_rep if pad_rows == 0 else np.vstack(
        [node_rep, np.zeros((pad_rows, _D), np.float32)])
    in_maps = []
    for k in range(_NCORES):
        m = {
            "rep": np.ascontiguousarray(rep_padded[k * shard : (k + 1) * shard]),
            "oidx": routing["oidx"][k],
        }
        if L:
            m.update(w_in)
            for l in range(L):
                m[f"seeds{l}"] = seeds[k][l]
        in_maps.append(m)

    import os

    from concourse import bass_utils
    try:
        res = bass_utils.run_bass_kernel_spmd(nc, in_maps, core_ids=list(range(_NCORES)))
    except ModuleNotFoundError:
        # BASS_TRACE set but the NTFF profiling shim is unavailable in this
        # environment - rerun with tracing disabled.
        os.environ["BASS_NEVER_TRACE"] = "1"
        res = bass_utils.run_bass_kernel_spmd(nc, in_maps, core_ids=list(range(_NCORES)))
    last_result = res

    out_flat = np.zeros((2 * S, _D), np.float32)
    for k in range(_NCORES):
        slots = routing["gl_slot"][k]
        if slots:
            rows = res.results[k]["out_gath"].reshape(128, G, _D)
            rows = rows.transpose(1, 0, 2).reshape(n_pad, _D)
            out_flat[np.asarray(slots)] = rows[: len(slots)]
        if len(routing["comp_slots"][k]):
            comp_v = res.results[k]["comp"]
            out_flat[routing["comp_slots"][k]] = comp_v[:, routing["comp_cols"][k]].T
    return out_flat.reshape(2, S, _D)



# revision 10
# speedup vs baseline: 1.6567x; 1.6567x over previous
"""DyGNN streaming-interaction kernel for Trainium2 (8 NeuronCores, Bass/Tile).

Strategy
--------
The reference is a sequential scan over S=2048 events touching rows of five
[N=100000, 128] node-state tables.  The output is only the PRE-update node
representation gathered at each event, so an event's update math matters only
if a LATER event reads one of its two nodes.  With random indices that is a
small set (~82 "relevant" events) with a very shallow dependency depth (2
levels for the expected data).

Host side (index math only): find relevant events, batch them into dependency
levels, compute operand provenance, and route the 2*S output-row gathers to
the core owning each node (node_rep is sharded row-wise across the 8 cores).

Device side (single SPMD program, per-core data), optimized for wall clock:
  * ONE multi-column indirect DMA gathers each core's share of output rows
    from its node_rep shard; one direct DMA writes them out.
  * The relevant-event recurrence (edge updaters + time-decayed LSTMs +
    combiner) runs as batched *bf16* matmuls (fp32 PSUM accumulate) in a
    transposed [feature, head-events | tail-events] layout, one batch per
    dependency level.  bf16 runs the PE at 4x the fp32 rate and halves
    LDWEIGHTS time; overall rel-l2 error stays ~6e-4.
  * The decay factor is shipped pre-computed from the host as exp(-dt)-1,
    so c_adj = c + cs*(exp(-dt)-1): no Exp on device, a single activation
    table set (sigmoid+tanh), no table-switch stalls.
  * Per level all four LSTM gate pre-activations accumulate in ONE PSUM
    bank and are activated with just two ACT instructions (sigmoid over
    i|f|o, tanh over g); edges and the short-term-memory tap share another
    bank and one tanh ACT.
  * Seeds and weights ship as one packed bf16 tensor staged with three
    coarse DMAs (sync + scalar queues) ordered by first use.

Host side assembles the [2, S, D] output from the per-core gather buffers
plus the computed representations for the few "patched" slots.
"""

import numpy as np

_NCORES = 8
_D = 128          # embedding dim == partition count
_MAXB = 64        # max events per device batch ([128, 8B] fp32 fits one PSUM bank)
_MAX_LEVELS = 64  # beyond this (adversarial chains) use the host fallback
_W_DECAY = 1.0

# operand order inside the packed per-level seed tile [128, 10*B]:
# RH RT DM1H DM1T | CH CT | HH HT | XHH XHT   (DM1 = exp(-w*dt) - 1).
_OP_RH, _OP_RT, _OP_DM1H, _OP_DM1T, _OP_CH, _OP_CT, _OP_HH, _OP_HT, _OP_XHH, _OP_XHT = range(10)
_N_OPS = 10

_cache = {}
last_result = None  # BassKernelResults of the most recent device run


def _preprocess(heads, tails, times):
    """Pure index/time analysis.  Returns None if the dependency structure is
    too deep for the compiled-levels approach (host fallback handles it).

    Level-1 events read only the initial tables and have no intra-level
    dependencies, so they are SPLIT across the 8 cores.  Level-1 events whose
    results feed later levels ("feeders"), and all events of level >= 2, are
    pinned to core 0 so every result-to-operand copy stays core-local (the
    SPMD program is identical on every core; the other cores run the deeper
    levels on zero padding and their results are ignored).
    """
    S = heads.shape[0]

    # -- pass 1 (backward): does any later event touch this event's nodes? --
    touched_later = np.zeros(S, dtype=bool)
    seen = {}
    for i in range(S - 1, -1, -1):
        h = int(heads[i]); t = int(tails[i])
        touched_later[i] = (h in seen) or (t in seen)
        seen[h] = True; seen[t] = True
    rel = [i for i in range(S) if touched_later[i]]

    # -- pass 2: dependency levels (global width cap keeps compile sane) --
    level_of = {}
    level_events = []
    last_level = {}
    for i in rel:
        h = int(heads[i]); t = int(tails[i])
        lv = max(last_level.get(h, 0), last_level.get(t, 0)) + 1
        while lv - 1 < len(level_events) and len(level_events[lv - 1]) >= _MAXB:
            lv += 1
        if lv > _MAX_LEVELS:
            return None
        while len(level_events) < lv:
            level_events.append([])
        level_of[i] = lv - 1
        level_events[lv - 1].append(i)
        last_level[h] = lv; last_level[t] = lv
    L = len(level_events)

    # -- pass 3 (forward): per-event operand sources (event-id based) --
    sources = {}   # i -> list of (op_idx, src_event|None, src_kind|table, node)
    dms = {}       # i -> (dm1_h, dm1_t) = exp(-w*dt) - 1
    lastw = {"rep": {}, "ch": {}, "hh": {}, "ct": {}, "ht": {}}
    last_time = {}
    feeders = set()
    for i in rel:
        h = int(heads[i]); t = int(tails[i]); tm = np.float32(times[i])
        srcs = []
        for op_idx, key, table, node in (
            (_OP_RH, "rep", "node_rep", h),
            (_OP_RT, "rep", "node_rep", t),
            (_OP_CH, "ch", "cell_head", h),
            (_OP_CT, "ct", "cell_tail", t),
            (_OP_HH, "hh", "hidden_head", h),
            (_OP_HT, "ht", "hidden_tail", t),
            (_OP_XHH, "hh", "hidden_head", t),
            (_OP_XHT, "ht", "hidden_tail", h),
        ):
            src = lastw[key].get(node)
            if src is not None:
                feeders.add(src[0])
            srcs.append((op_idx, src, table, node))
        sources[i] = srcs
        dt_h = np.float32(tm - np.float32(last_time.get(h, 0.0)))
        dt_t = np.float32(tm - np.float32(last_time.get(t, 0.0)))
        dms[i] = (np.float32(np.exp(np.float32(-_W_DECAY) * dt_h, dtype=np.float32) - np.float32(1.0)),
                  np.float32(np.exp(np.float32(-_W_DECAY) * dt_t, dtype=np.float32) - np.float32(1.0)))
        lastw["rep"][h] = (i, "NRH")
        lastw["rep"][t] = (i, "NRT")
        lastw["ch"][h] = (i, "CHN")
        lastw["hh"][h] = (i, "HHN")
        lastw["ct"][t] = (i, "CTN")
        lastw["ht"][t] = (i, "HTN")
        last_time[h] = tm; last_time[t] = tm

    # -- pass 4: core assignment.  Feeders + all level>=2 events -> core 0;
    # remaining level-1 events spread greedily across all cores. --
    chunks = [[[] for _ in range(L)] for _ in range(_NCORES)]  # [core][lv]
    assign = {}
    free_l1 = []
    for i in rel:
        lv = level_of[i]
        if lv > 0 or i in feeders:
            assign[i] = (0, lv, len(chunks[0][lv]))
            chunks[0][lv].append(i)
        else:
            free_l1.append(i)
    if L:
        loads = [len(chunks[k][0]) for k in range(_NCORES)]
        for i in free_l1:
            k = int(np.argmin(loads))
            assign[i] = (k, 0, len(chunks[k][0]))
            chunks[k][0].append(i)
            loads[k] += 1

    Bs = [max(len(chunks[k][l]) for k in range(_NCORES)) for l in range(L)]
    off = [0]
    for b in Bs:
        off.append(off[-1] + 2 * b)
    Ctot = off[-1]

    # -- pass 5: program copies (core-0 positions; identical on all cores)
    # and per-core seed fills.  Copy sources address the v2 result tiles:
    # "R" = R_new [2B], "C" = C_new [2B], "H" = the U tile [4B] whose
    # cols [0:B] hold h_h and [3B:4B] hold h_t. --
    copies = [[] for _ in range(L)]
    seed_fill = [[[] for _ in range(L)] for _ in range(_NCORES)]
    dm_fill = [[[] for _ in range(L)] for _ in range(_NCORES)]
    for i in rel:
        k, lv, p = assign[i]
        B = Bs[lv]
        for (op_idx, src, table, node) in sources[i]:
            dst_col = op_idx * B + p
            if src is not None:
                j, skind = src
                sk, slv, sp = assign[j]
                # feeders and deep levels are all on core 0, as is event i
                assert sk == 0 and k == 0
                sB = Bs[slv]
                stile, s_col = {
                    "CHN": ("C", sp), "CTN": ("C", sB + sp),
                    "HHN": ("H", sp), "HTN": ("H", 3 * sB + sp),
                    "NRH": ("R", sp), "NRT": ("R", sB + sp),
                }[skind]
                copies[lv].append((dst_col, slv, stile, s_col))
            else:
                seed_fill[k][lv].append((dst_col, table, node))
        dm_fill[k][lv].append((p, dms[i][0], dms[i][1]))

    return {
        "touched_later": touched_later,
        "assign": assign,
        "Bs": Bs,
        "off": off,
        "Ctot": Ctot,
        "copies": copies,
        "seed_fill": seed_fill,
        "dm_fill": dm_fill,
    }


def _route_outputs(heads, tails, plan, N):
    """Route each of the 2*S output slots to either a per-core gather or a
    computed-rep column."""
    S = heads.shape[0]
    Bs, off, assign = plan["Bs"], plan["off"], plan["assign"]
    touched_later = plan["touched_later"]
    shard = -(-N // _NCORES)  # ceil

    gl_idx = [[] for _ in range(_NCORES)]
    gl_slot = [[] for _ in range(_NCORES)]
    comp_slots = [[] for _ in range(_NCORES)]  # per compute-owner core
    comp_cols = [[] for _ in range(_NCORES)]
    lastw_rep_col = {}
    for i in range(S):
        h = int(heads[i]); t = int(tails[i])
        for role, n in ((0, h), (1, t)):
            slot = role * S + i
            cc = lastw_rep_col.get(n)
            if cc is not None:
                comp_slots[cc[0]].append(slot); comp_cols[cc[0]].append(cc[1])
            else:
                k = n // shard
                gl_idx[k].append(n - k * shard)
                gl_slot[k].append(slot)
        if touched_later[i]:
            k, lv, p = assign[i]
            lastw_rep_col[h] = (k, off[lv] + p)           # NRH column
            lastw_rep_col[t] = (k, off[lv] + Bs[lv] + p)  # NRT column
    max_load = max(len(x) for x in gl_idx)
    G = max(1, -(-max_load // 128))
    n_pad = G * 128
    oidx = []
    for k in range(_NCORES):
        a = np.zeros(n_pad, dtype=np.int32)
        a[: len(gl_idx[k])] = gl_idx[k]
        # gathered row g*128+p comes from SBUF [p, g*128:(g+1)*128]
        oidx.append(np.ascontiguousarray(a.reshape(G, 128).T))
    return {
        "shard": shard,
        "G": G,
        "n_pad": n_pad,
        "oidx": oidx,
        "gl_slot": gl_slot,
        "comp_slots": [np.array(x, dtype=np.int64) for x in comp_slots],
        "comp_cols": [np.array(x, dtype=np.int64) for x in comp_cols],
    }


# packed weight layout, ordered by first use (each entry is a 128-col tile)
_WNAMES = ("Weh1", "Weh2", "Wet1", "Wet2", "Wdh", "Wdt") + tuple(
    f"{m}{q}" for q in range(4) for m in ("Whh", "Wht", "Wxh", "Wxt")
) + ("Wc1", "Wc2")


def _build_program(shard, G, Bs, copies, Ctot):
    from contextlib import ExitStack

    import concourse.bacc as bacc
    import concourse.bass as bass
    import concourse.tile as tile
    from concourse import mybir

    f32 = mybir.dt.float32
    bf16 = mybir.dt.bfloat16
    i32 = mybir.dt.int32
    AFT = mybir.ActivationFunctionType

    nc = bacc.Bacc(
        "TRN2",
        debug=False,
        enable_asserts=False,
        target_bir_lowering=False,
        num_devices=_NCORES,
        enable_partition_id=False,
    )

    rep = nc.dram_tensor("rep", [shard, _D], f32, kind="ExternalInput").ap()
    oidx = nc.dram_tensor("oidx", [128, G], i32, kind="ExternalInput").ap()
    out_gath = nc.dram_tensor("out_gath", [128, G * _D], f32, kind="ExternalOutput").ap()

    L = len(Bs)
    seed_off = []
    o = 0
    for B in Bs:
        seed_off.append(o)
        o += _N_OPS * B
    seeds_end = o
    woffs = {}
    for n in _WNAMES:
        woffs[n] = o
        o += _D
    PC = o
    comp = None
    if L:
        pack = nc.dram_tensor("pack", [128, PC], bf16, kind="ExternalInput").ap()
        comp = nc.dram_tensor("comp", [128, Ctot], bf16, kind="ExternalOutput").ap()

    with tile.TileContext(nc) as tc, ExitStack() as ctx:
        sp = ctx.enter_context(tc.tile_pool(name="s", bufs=1))
        tp = ctx.enter_context(tc.tile_pool(name="tmp", bufs=3))
        pp = ctx.enter_context(tc.tile_pool(name="ps", bufs=8, space="PSUM"))

        # A dummy sigmoid on scratch data forces the sigmoid_and_others
        # activation table set (which also holds tanh) to load ONCE, at
        # program start, overlapped with input staging - otherwise the
        # 1.28us table switch lands mid-recurrence at the first real
        # sigmoid.
        if L:
            dum = sp.tile([128, 1], f32, tag="dum", name="dum")
            nc.vector.memset(dum[:], 0.0)
            nc.scalar.activation(dum[:], dum[:], AFT.Sigmoid)

        # ---- output-row gather: ONE indirect DMA, issued first ----
        idx_sb = sp.tile([128, G], i32, tag="idx", name="idx_sb")
        nc.sync.dma_start(idx_sb[:], oidx[:])
        # HW indirect DMA gathers one row per partition per instruction
        # (the offset vector is one element per partition), so issue G of
        # them; they all overlap the staging DMAs and the recurrence.
        gt = sp.tile([128, G * _D], f32, tag="gath", name="gt")
        for g in range(G):
            nc.gpsimd.indirect_dma_start(
                out=gt[:, g * _D : (g + 1) * _D],
                out_offset=None,
                in_=rep[:],
                in_offset=bass.IndirectOffsetOnAxis(ap=idx_sb[:, g : g + 1], axis=0),
            )

        if L:
            # ---- staging: 3 coarse DMAs from the packed tensor, by first use
            pk = sp.tile([128, PC], bf16, tag="pk", name="pk")
            stageA = woffs["Whh0"]   # seeds + edge/cs weights
            stageB = woffs["Wc1"]    # gate weights
            nc.sync.dma_start(pk[:, :stageA], pack[:, :stageA])
            nc.sync.dma_start(pk[:, stageA:stageB], pack[:, stageA:stageB])
            nc.sync.dma_start(pk[:, stageB:], pack[:, stageB:])

            def w(name):
                wo = woffs[name]
                return pk[:, wo : wo + _D]

        # out-write emitted after the staging DMAs so its semaphore wait on
        # the gather can't delay the staging configs on the sync queue
        nc.sync.dma_start(out_gath[:], gt[:])

        # ---- relevant-event recurrence ----
        results = []
        for l, B in enumerate(Bs) if L else []:
            B2 = 2 * B
            SD = pk[:, seed_off[l] : seed_off[l] + _N_OPS * B]
            blk = lambda op: SD[:, op * B : (op + 1) * B]
            pair = lambda op: SD[:, op * B : (op + 2) * B]
            RH, RT = blk(_OP_RH), blk(_OP_RT)
            DM1 = pair(_OP_DM1H)
            C2 = pair(_OP_CH)
            HH, HT = blk(_OP_HH), blk(_OP_HT)
            XH2 = pair(_OP_XHH)

            # operand copies from earlier levels (R sources are produced
            # latest, keep them last and on the faster DVE; C/H on gpsimd)
            for (dst_col, slv, stile, s_col) in copies[l]:
                eng = nc.vector if stile == "R" else nc.gpsimd
                eng.tensor_copy(
                    SD[:, dst_col : dst_col + 1],
                    results[slv][stile][:, s_col : s_col + 1],
                )

            def tmp(tag, cols, dt=bf16):
                return tp.tile([128, cols], dt, tag=tag, name=f"t{l}_{tag}")

            # combiner operand tile U = [h_h | hh[t] | ht[h] | h_t]
            U = sp.tile([128, 4 * B], bf16, tag=f"res{l}_U", name=f"res{l}_U")
            nc.vector.tensor_copy(U[:, B : 3 * B], XH2)

            # edges+cs bank: [edge_h | edge_t | cs_h | cs_t]
            psA = pp.tile([128, 4 * B], f32, tag="ps", name=f"psA{l}")
            nc.tensor.matmul(psA[:, 0:B], w("Weh1"), RH, start=True, stop=False)
            nc.tensor.matmul(psA[:, 0:B], w("Weh2"), RT, start=False, stop=True)
            nc.tensor.matmul(psA[:, B:B2], w("Wet1"), RH, start=True, stop=False)
            nc.tensor.matmul(psA[:, B:B2], w("Wet2"), RT, start=False, stop=True)
            nc.tensor.matmul(psA[:, B2 : B2 + B], w("Wdh"), C2[:, 0:B], start=True, stop=True)
            nc.tensor.matmul(psA[:, B2 + B :], w("Wdt"), C2[:, B:B2], start=True, stop=True)
            EGCS = tmp("egcs", 4 * B)
            nc.scalar.activation(EGCS[:], psA[:], AFT.Tanh)
            EG = EGCS[:, 0:B2]
            CS = EGCS[:, B2:]

            # gate bank: [i | f | o | g], each gate [head | tail].
            # Wh-halves first: they depend only on seeds (and early copies),
            # so the PE can run them while the edge activation is in flight.
            # One PSUM bank for all four gates.  start=True zeroes the WHOLE
            # bank (has_written cleared bank-wide), so only the very first
            # matmul starts; every later matmul uses start=False and the
            # per-element has_written bit makes it a first-write (overwrite)
            # or an accumulate as appropriate.  The sim's bank-granular group
            # check can't express this, hence skip_group_check.
            psB = pp.tile([128, 8 * B], f32, tag="ps", name=f"psB{l}")
            for q in range(4):
                nc.tensor.matmul(psB[:, 2 * q * B : (2 * q + 1) * B],
                                 w(f"Whh{q}"), HH, start=(q == 0), stop=False,
                                 skip_group_check=True)
                nc.tensor.matmul(psB[:, (2 * q + 1) * B : (2 * q + 2) * B],
                                 w(f"Wht{q}"), HT, start=False, stop=False,
                                 skip_group_check=True)
            for q in range(4):
                nc.tensor.matmul(psB[:, 2 * q * B : (2 * q + 1) * B],
                                 w(f"Wxh{q}"), EG[:, 0:B], start=False, stop=False,
                                 skip_group_check=True)
                nc.tensor.matmul(psB[:, (2 * q + 1) * B : (2 * q + 2) * B],
                                 w(f"Wxt{q}"), EG[:, B:B2], start=False,
                                 stop=(q == 3), skip_group_check=True)
            GS = tmp("gs", 8 * B)
            nc.scalar.activation(GS[:, 0 : 6 * B], psB[:, 0 : 6 * B], AFT.Sigmoid)
            nc.scalar.activation(GS[:, 6 * B :], psB[:, 6 * B :], AFT.Tanh)
            gi, gf, go, gg = (GS[:, 2 * q * B : (2 * q + 2) * B] for q in range(4))

            # c_adj = c + cs*(exp(-dt)-1); c_new = f*c_adj + i*g
            CSD = tmp("csd", B2)
            nc.vector.tensor_mul(CSD[:], CS, DM1)
            CADJ = tmp("cadj", B2)
            nc.vector.tensor_add(CADJ[:], C2, CSD[:])
            FC = tmp("fc", B2)
            nc.vector.tensor_mul(FC[:], gf, CADJ[:])
            IG = tmp("ig", B2)
            nc.vector.tensor_mul(IG[:], gi, gg)
            C_new = sp.tile([128, B2], bf16, tag=f"res{l}_C", name=f"res{l}_C")
            nc.vector.tensor_add(C_new[:], FC[:], IG[:])
            TC = tmp("tc", B2)
            nc.scalar.activation(TC[:], C_new[:], AFT.Tanh)
            # h halves written straight into the combiner operand tile
            nc.vector.tensor_mul(U[:, 0:B], go[:, 0:B], TC[:, 0:B])
            nc.vector.tensor_mul(U[:, 3 * B :], go[:, B:B2], TC[:, B:B2])

            psC = pp.tile([128, B2], f32, tag="ps", name=f"psC{l}")
            nc.tensor.matmul(psC[:], w("Wc1"), U[:, 0:B2], start=True, stop=False)
            nc.tensor.matmul(psC[:], w("Wc2"), U[:, B2:], start=False, stop=True)
            R_new = sp.tile([128, B2], bf16, tag=f"res{l}_R", name=f"res{l}_R")
            nc.scalar.activation(R_new[:], psC[:], AFT.Tanh)
            results.append({"C": C_new, "H": U, "R": R_new})

            o0 = sum(2 * b for b in Bs[:l])
            nc.sync.dma_start(comp[:, o0 : o0 + B2], R_new[:])

    nc.compile()
    return nc


def _pack_inputs(inputs, Bs, seed_fill, dm_fill, core):
    """Per-core packed bf16 tensor: [seeds per level | 24 weight tiles]."""
    from ml_dtypes import bfloat16

    f32 = np.float32
    tables = {k: np.asarray(inputs[k], f32) for k in (
        "node_rep", "cell_head", "hidden_head", "cell_tail", "hidden_tail")}
    cols = sum(_N_OPS * B for B in Bs) + len(_WNAMES) * _D
    out = np.zeros((128, cols), f32)
    o = 0
    for l, B in enumerate(Bs):
        for (col, tab, node) in seed_fill[core][l]:
            out[:, o + col] = tables[tab][node]
        for (p, dm_h, dm_t) in dm_fill[core][l]:
            out[:, o + _OP_DM1H * B + p] = dm_h
            out[:, o + _OP_DM1T * B + p] = dm_t
        o += _N_OPS * B
    for n in _WNAMES:
        if n[-1] in "0123" and n[:-1] in ("Wxh", "Whh", "Wxt", "Wht"):
            q = int(n[-1])
            out[:, o : o + _D] = np.asarray(inputs[n[:-1]], f32)[:, q * 128 : (q + 1) * 128]
        else:
            out[:, o : o + _D] = np.asarray(inputs[n], f32)
        o += _D
    return out.astype(bfloat16)


def _numpy_fallback(heads, tails, times, node_rep, cell_head, hidden_head,
                    cell_tail, hidden_tail, Weh1, Weh2, beh, Wet1, Wet2, bet,
                    Wxh, Whh, bh, Wdh, bdh, Wxt, Wht, bt, Wdt, bdt, Wc1, Wc2):
    """Exact float32 reference semantics; safety net for pathological inputs."""
    f32 = np.float32
    S = heads.shape[0]
    D = node_rep.shape[1]
    rep = np.array(node_rep, f32); ch = np.array(cell_head, f32)
    hh = np.array(hidden_head, f32); ct = np.array(cell_tail, f32)
    ht = np.array(hidden_tail, f32)
    rt = np.zeros(node_rep.shape[0], f32)
    out = np.zeros((2, S, D), f32)

    def sig(x):
        return f32(1.0) / (f32(1.0) + np.exp(-x, dtype=f32))

    def tlstm(x, c, h, dec, Wx, Wh, b, Wd, bd):
        cs = np.tanh(c @ Wd + bd, dtype=f32)
        c_adj = c - cs + cs * dec
        z = x @ Wx + h @ Wh + b
        i, f, o, g = np.split(z, 4)
        i = sig(i); f = sig(f); o = sig(o); g = np.tanh(g, dtype=f32)
        c_new = f * c_adj + i * g
        return c_new, o * np.tanh(c_new, dtype=f32)

    for j in range(S):
        h_i = int(heads[j]); t_i = int(tails[j]); tm = f32(times[j])
        rep_h = rep[h_i].copy(); rep_t = rep[t_i].copy()
        out[0, j] = rep_h; out[1, j] = rep_t
        dec_h = np.exp(f32(-_W_DECAY) * (tm - rt[h_i]), dtype=f32)
        dec_t = np.exp(f32(-_W_DECAY) * (tm - rt[t_i]), dtype=f32)
        edge_h = np.tanh(rep_h @ Weh1 + rep_t @ Weh2 + beh, dtype=f32)
        edge_t = np.tanh(rep_h @ Wet1 + rep_t @ Wet2 + bet, dtype=f32)
        c_h, h_h = tlstm(edge_h, ch[h_i], hh[h_i], dec_h, Wxh, Whh, bh, Wdh, bdh)
        c_t, h_t = tlstm(edge_t, ct[t_i], ht[t_i], dec_t, Wxt, Wht, bt, Wdt, bdt)
        new_rep_h = np.tanh(h_h @ Wc1 + ht[h_i] @ Wc2, dtype=f32)
        new_rep_t = np.tanh(hh[t_i] @ Wc1 + h_t @ Wc2, dtype=f32)
        rep[h_i] = new_rep_h; rep[t_i] = new_rep_t
        ch[h_i] = c_h; hh[h_i] = h_h; ct[t_i] = c_t; ht[t_i] = h_t
        rt[h_i] = tm; rt[t_i] = tm
    return out


def kernel(**inputs):
    global last_result
    heads = np.asarray(inputs["heads"]).astype(np.int64)
    tails = np.asarray(inputs["tails"]).astype(np.int64)
    times = np.asarray(inputs["times"], dtype=np.float32)
    node_rep = np.asarray(inputs["node_rep"], dtype=np.float32)
    N = node_rep.shape[0]
    S = heads.shape[0]

    plan = _preprocess(heads, tails, times)
    has_bias = any(
        np.any(np.asarray(inputs[k], np.float32))
        for k in ("beh", "bet", "bdh", "bdt", "bh", "bt"))
    if plan is None or has_bias:
        return _numpy_fallback(
            heads, tails, times,
            *[np.asarray(inputs[k], np.float32) for k in (
                "node_rep", "cell_head", "hidden_head", "cell_tail", "hidden_tail",
                "Weh1", "Weh2", "beh", "Wet1", "Wet2", "bet",
                "Wxh", "Whh", "bh", "Wdh", "bdh", "Wxt", "Wht", "bt", "Wdt", "bdt",
                "Wc1", "Wc2")],
        )

    routing = _route_outputs(heads, tails, plan, N)
    shard, G, n_pad = routing["shard"], routing["G"], routing["n_pad"]
    Bs, Ctot = plan["Bs"], plan["Ctot"]
    L = len(Bs)

    sig = (shard, G, tuple(Bs),
           tuple(tuple(c) for lc in plan["copies"] for c in lc), Ctot)
    nc = _cache.get(sig)
    if nc is None:
        nc = _build_program(shard, G, Bs, plan["copies"], Ctot)
        _cache[sig] = nc

    pad_rows = shard * _NCORES - N
    rep_padded = node_rep if pad_rows == 0 else np.vstack(
        [node_rep, np.zeros((pad_rows, _D), np.float32)])
    in_maps = []
    for k in range(_NCORES):
        m = {
            "rep": np.ascontiguousarray(rep_padded[k * shard : (k + 1) * shard]),
            "oidx": routing["oidx"][k],
        }
        if L:
            m["pack"] = _pack_inputs(inputs, Bs, plan["seed_fill"], plan["dm_fill"], k)
        in_maps.append(m)

    import os

    from concourse import bass_utils
    try:
        res = bass_utils.run_bass_kernel_spmd(nc, in_maps, core_ids=list(range(_NCORES)))
    except ModuleNotFoundError:
        # BASS_TRACE set but the NTFF profiling shim is unavailable in this
        # environment - rerun with tracing disabled.
        os.environ["BASS_NEVER_TRACE"] = "1"
        res = bass_utils.run_bass_kernel_spmd(nc, in_maps, core_ids=list(range(_NCORES)))
    last_result = res

    out_flat = np.zeros((2 * S, _D), np.float32)
    for k in range(_NCORES):
        slots = routing["gl_slot"][k]
        if slots:
            rows = res.results[k]["out_gath"].reshape(128, G, _D)
            rows = rows.transpose(1, 0, 2).reshape(n_pad, _D)
            out_flat[np.asarray(slots)] = rows[: len(slots)]
        if len(routing["comp_slots"][k]):
            comp_v = np.asarray(res.results[k]["comp"], np.float32)
            out_flat[routing["comp_slots"][k]] = comp_v[:, routing["comp_cols"][k]].T
    return out_flat.reshape(2, S, _D)
